# revision 16
# baseline (speedup 1.0000x reference)
"""Causal attention with bias for B=2,H=16,N=2048,D=128 on 8 trn2 NeuronCores.

Sharding: core c handles heads {2c, 2c+1} for both batches (head-parallel).

Algorithm (v4, ACT-bound design):
  exp(s + bias) = exp(s) * exp(bias), with exp(bias) precomputed on the host
  (zeros above the diagonal double as the causal mask). Device per tile:
    PE:  S^T[j,i] = kT^T q  (bf16, q pre-scaled)      -> PSUM f32
    ACT: exp(S^T)                                     -> SBUF bf16
    DVE: attn = exp(S^T) * expb   (bf16, in-place)
    PE:  PV against [v | ones]  (denominator rides in column D)
    DVE: po (f32 PSUM) -> bf16 staging
  numerator/denominator division happens on the HOST (fp32), so no
  reciprocal / normalize on device.

  The scalar engine is the bottleneck (~8.9e6 exps/core at 1 elem/cycle +
  ~280ns/instr, capped at 1024-elem tiles by the 8-bank PSUM); the schedule
  keeps ACT streaming: PV of the previous chunk is interleaved between the
  QK pairs of the current chunk, head 1 runs its chunks in descending order
  so the drain tail is minimal, and DMA issues are merged into few large
  transfers so the sync sequencer never backs up.
"""

import os

import numpy as np
import ml_dtypes

import concourse.bass as bass
import concourse.bacc as bacc
import concourse.mybir as mybir
import concourse.tile as tile
from concourse.bass_utils import run_bass_kernel_spmd

B, H, N, D = 2, 16, 2048, 128
NCORES = 8
HPC = H // NCORES          # heads per core
SCALE = float(D) ** -0.5
CHUNK = 512                # i-chunk width (one psum bank of fp32)
JB = 128                   # j block (partition dim of S^T tiles)
NCHUNK = N // CHUNK        # 4
JPC = CHUNK // JB          # j blocks per chunk: 4
HALF = N // 2
JPH = HALF // JB           # j blocks per v half-tile: 8

F32 = mybir.dt.float32
BF16 = mybir.dt.bfloat16

# diag pack segment offsets for k=0..3 (widths 512,384,256,128)
DSEG = [0, 512, 896, 1152]
DW = [512, 384, 256, 128]
DPACK = 1280

PASSES_OFF = set(
    p for p in os.environ.get("ATTN_PASSES_OFF", "").split(",") if p
)


class PatchedBacc(bacc.Bacc):
    """Bacc with individually disableable scheduling passes (race bisection)."""

    def move_matmul_waits_to_ldweights(self):
        if "nomm" not in PASSES_OFF:
            super().move_matmul_waits_to_ldweights()

    def replace_nops_with_events(self):
        if "noevt" not in PASSES_OFF:
            super().replace_nops_with_events()

    def fuse_nops(self, engine):
        if "nofuse" not in PASSES_OFF:
            super().fuse_nops(engine)

    def fuse_regops(self):
        if "noregfuse" not in PASSES_OFF:
            super().fuse_regops()


def build_nc():
    nc = PatchedBacc(None, target_bir_lowering=False)

    qT_d = nc.dram_tensor("qT", [B, HPC, D, N], BF16, kind="ExternalInput").ap()
    kT_d = nc.dram_tensor("kT", [B, HPC, D, N], BF16, kind="ExternalInput").ap()
    # v with ones column, partition-major, halves merged: [b, h, p, half, jb, d+1]
    vp_d = nc.dram_tensor(
        "vp", [B, HPC, JB, 2, JPH, D + 1], BF16, kind="ExternalInput"
    ).ap()
    # exp(bias^T) full matrix (zeros above diagonal), natural [h, j, i]
    ebF_d = nc.dram_tensor("ebF", [HPC, N, N], BF16, kind="ExternalInput").ap()
    # exp(bias^T) diag blocks, packed per chunk: [h, c, p, 1280]
    ebD_d = nc.dram_tensor(
        "ebD", [HPC, NCHUNK, JB, DPACK], BF16, kind="ExternalInput"
    ).ap()
    # numerator | denominator staging: [h, c, p, b*4*(D+1)]
    out_d = nc.dram_tensor(
        "out", [HPC, NCHUNK, JB, B * JPC * (D + 1)], BF16, kind="ExternalOutput"
    ).ap()

    with tile.TileContext(nc) as tc:
        with (
            tc.tile_pool(name="singles", bufs=1) as singles,
            tc.tile_pool(name="kq", bufs=4) as kq_pool,
            tc.tile_pool(name="vp", bufs=4) as v_pool,
            tc.tile_pool(name="ebq", bufs=2) as ebq_pool,
            tc.tile_pool(name="ebd", bufs=3) as ebd_pool,
            tc.tile_pool(name="attn", bufs=28) as attn_pool,
            tc.tile_pool(name="stage", bufs=3) as stage_pool,
            tc.tile_pool(name="ps", bufs=3, space="PSUM") as ps_pool,
            tc.tile_pool(name="po", bufs=2, space="PSUM") as po_pool,
        ):
            kq_t, v_t = {}, {}

            # ---- loads ----------------------------------------------------

            def load_kq_small(hi, b, eng):
                """chunk-0 columns of qT/kT: fast-start tiles."""
                for which, src in (("q", qT_d), ("k", kT_d)):
                    t = kq_pool.tile(
                        [D, CHUNK], BF16, tag="kq0", name=f"{which}0_t"
                    )
                    eng.dma_start(out=t[:], in_=src[b, hi, :, 0:CHUNK])
                    kq_t[(which, hi, b, "c0")] = t

            def load_kq_rest(hi, b):
                """columns 512:2048 of qT/kT for head 0."""
                for which, src in (("q", qT_d), ("k", kT_d)):
                    t = kq_pool.tile(
                        [D, N - CHUNK], BF16, tag="kqr", name=f"{which}r_t"
                    )
                    nc.sync.dma_start(out=t[:], in_=src[b, hi, :, CHUNK:N])
                    kq_t[(which, hi, b, "rest")] = t

            def load_kq_full(hi, b):
                """whole rows of qT/kT for head 1."""
                for which, src in (("q", qT_d), ("k", kT_d)):
                    t = kq_pool.tile([D, N], BF16, tag="kqf", name=f"{which}f_t")
                    nc.sync.dma_start(out=t[:], in_=src[b, hi, :, :])
                    kq_t[(which, hi, b, "full")] = t

            def kq_col(which, hi, b, col0, width):
                """[D, width] slice at global column col0."""
                t = kq_t.get((which, hi, b, "full"))
                if t is not None:
                    return t[:, col0 : col0 + width]
                if col0 < CHUNK:
                    assert col0 + width <= CHUNK
                    return kq_t[(which, hi, b, "c0")][:, col0 : col0 + width]
                return kq_t[(which, hi, b, "rest")][
                    :, col0 - CHUNK : col0 - CHUNK + width
                ]

            def kT_sl(hi, b, jb):
                return kq_col("k", hi, b, jb * JB, JB)

            def qT_sl(hi, b, c, off=0):
                return kq_col("q", hi, b, c * CHUNK + off, CHUNK - off)

            def load_v(hi, b):
                t = v_pool.tile([JB, 2, JPH, D + 1], BF16, tag="v", name="v_t")
                nc.sync.dma_start(out=t[:], in_=vp_d[b, hi])
                v_t[(hi, b)] = t

            def v_sl(hi, b, jb):
                return v_t[(hi, b)][:, jb // JPH, jb % JPH, :]

            ebq_tiles, ebd_tiles = {}, {}

            def load_ebq(hi, c, eng=None):
                """full-region expb for chunk (hi, c): one DMA, 4c j-blocks."""
                if c == 0:
                    return
                i0 = c * CHUNK
                t = ebq_pool.tile(
                    [JB, 4 * NCHUNK - 4, CHUNK], BF16, tag="ebq", name="ebq_t"
                )
                (eng or nc.sync).dma_start(
                    out=t[:, 0 : 4 * c, :],
                    in_=ebF_d[hi, 0 : c * CHUNK, i0 : i0 + CHUNK].rearrange(
                        "(t p) i -> p t i", p=JB
                    ),
                )
                ebq_tiles[(hi, c)] = t

            def load_ebd(hi, c, eng=None):
                t = ebd_pool.tile([JB, DPACK], BF16, tag="ebd", name="ebd_t")
                (eng or nc.sync).dma_start(out=t[:], in_=ebD_d[hi, c])
                ebd_tiles[(hi, c)] = t

            # ---- per-(hi, chunk) work units -------------------------------

            def qk_pair(hi, b, c, g, attn_full):
                """Full pair g (jb = 2g, 2g+1): QK -> exp -> *expb."""
                ps = ps_pool.tile([JB, 2 * CHUNK], F32, tag="ps", name="ps_t")
                for t in range(2):
                    sl = slice(t * CHUNK, (t + 1) * CHUNK)
                    nc.tensor.matmul(
                        ps[:, sl],
                        lhsT=kT_sl(hi, b, 2 * g + t),
                        rhs=qT_sl(hi, b, c),
                        start=True,
                        stop=True,
                    )
                at = attn_pool.tile([JB, 2 * CHUNK], BF16, tag="attn", name="at_t")
                nc.scalar.activation(
                    at[:], ps[:], mybir.ActivationFunctionType.Exp
                )
                eb = ebq_tiles[(hi, c)][:, 2 * g : 2 * g + 2, :]
                nc.vector.tensor_mul(
                    at[:], at[:], eb.rearrange("p t i -> p (t i)")
                )
                attn_full[(b, g)] = at

            def qk_diag(hi, b, c, pair, attn_diag):
                """Diag pair (k = 2*pair, 2*pair+1): narrowed QK, exp, *expb."""
                ebd = ebd_tiles[(hi, c)]
                ps = ps_pool.tile([JB, 2 * CHUNK], F32, tag="ps", name="ps_t")
                for t in range(2):
                    k = 2 * pair + t
                    off = k * JB
                    nc.tensor.matmul(
                        ps[:, t * CHUNK + off : (t + 1) * CHUNK],
                        lhsT=kT_sl(hi, b, JPC * c + k),
                        rhs=qT_sl(hi, b, c, off),
                        start=True,
                        stop=True,
                    )
                at = attn_pool.tile([JB, 2 * CHUNK], BF16, tag="attn", name="at_t")
                if pair == 0:
                    # k=0 is full width; k=1 wastes only 128 cols: one big
                    # activation beats two narrowed ones (fixed ~280ns/instr)
                    nc.scalar.activation(
                        at[:], ps[:], mybir.ActivationFunctionType.Exp
                    )
                else:
                    for t in range(2):
                        k = 2 * pair + t
                        off = k * JB
                        sl = slice(t * CHUNK + off, (t + 1) * CHUNK)
                        nc.scalar.activation(
                            at[:, sl], ps[:, sl],
                            mybir.ActivationFunctionType.Exp,
                        )
                for t in range(2):
                    k = 2 * pair + t
                    off = k * JB
                    sl = slice(t * CHUNK + off, (t + 1) * CHUNK)
                    nc.vector.tensor_mul(
                        at[:, sl], at[:, sl],
                        ebd[:, DSEG[k] : DSEG[k] + DW[k]],
                    )
                attn_diag[(b, pair)] = at

            def attn_slice(b, c, jb, sub, attn_full, attn_diag):
                if jb < JPC * c:
                    t = attn_full[(b, jb // 2)]
                    o = (jb % 2) * CHUNK
                else:
                    k = jb - JPC * c
                    t = attn_diag[(b, k // 2)]
                    o = (k % 2) * CHUNK
                return t[:, o + sub * JB : o + (sub + 1) * JB]

            def pv_unit(hi, b, c, sub, state):
                """PV accumulation for output block ib = 4c+sub of (b, c)."""
                ib = JPC * c + sub
                po = po_pool.tile([JB, D + 1], F32, tag="po", name="po_t")
                af, ad = state["attn"][(hi, c)]
                for jb in range(ib + 1):
                    nc.tensor.matmul(
                        po[:],
                        lhsT=attn_slice(b, c, jb, sub, af, ad),
                        rhs=v_sl(hi, b, jb),
                        start=(jb == 0),
                        stop=(jb == ib),
                    )
                stg = state["stg"].get((hi, c))
                if stg is None:
                    stg = stage_pool.tile(
                        [JB, B * JPC * (D + 1)], BF16, tag="stg", name="stg_t"
                    )
                    state["stg"][(hi, c)] = stg
                o = (b * JPC + sub) * (D + 1)
                nc.vector.tensor_copy(out=stg[:, o : o + (D + 1)], in_=po[:])
                if sub == JPC - 1 and b == B - 1:
                    nc.sync.dma_start(out=out_d[hi, c], in_=stg[:])

            # ---- main schedule -------------------------------------------
            # head 0 ascending, head 1 descending: the final chunk is then
            # (1, 0), whose PV drain is the cheapest possible tail
            seq = [(0, 0), (0, 1), (0, 2), (0, 3), (1, 3), (1, 2), (1, 1), (1, 0)]

            state = {"attn": {}, "stg": {}}
            prev_pv = None

            for ti, (hi, c) in enumerate(seq):
                if ti == 0:
                    # DMA transfers effectively serialize in issue order, so
                    # startup criticals go first, all on the scalar engine's
                    # queue (its preamble finishes ~3us before sync's):
                    # chunk-0 q/k tiles, then the first chunk's expb.
                    for b in range(B):
                        load_kq_small(0, b, nc.scalar)
                    # warm-up exp AFTER the DMA issues so it doesn't block
                    warm = singles.tile([JB, 1], F32, tag="warm", name="warm")
                    nc.vector.memset(warm[:], 0.0)
                    nc.scalar.activation(
                        warm[:], warm[:], mybir.ActivationFunctionType.Exp
                    )
                    load_ebd(0, 0, eng=nc.scalar)
                    load_ebq(0, 1, eng=nc.scalar)
                    load_ebd(0, 1, eng=nc.scalar)
                    # gate the sync queue behind the scalar-issued criticals:
                    # a tiny SBUF->SBUF copy depending on the last kq tile
                    # keeps sync's (big, less urgent) transfers from stealing
                    # DMA bandwidth during startup
                    gate = singles.tile([1, 2], BF16, tag="gate", name="gate")
                    nc.sync.dma_start(
                        out=gate[:], in_=kq_t[("k", 0, 1, "c0")][0:1, 0:2]
                    )
                    # remaining loads in need-order on sync
                    for b in range(B):
                        load_kq_rest(0, b)
                    for b in range(B):
                        load_v(0, b)
                elif ti == 1:
                    load_ebq(0, 2)
                    load_ebd(0, 2)
                    for b in range(B):
                        load_kq_full(1, b)
                elif ti == 2:
                    load_ebq(0, 3)
                    load_ebd(0, 3)
                    for b in range(B):
                        load_v(1, b)
                elif ti == 3:
                    load_ebq(1, 3)
                    load_ebd(1, 3)
                elif ti == 4:
                    load_ebq(1, 2)
                    load_ebd(1, 2)
                elif ti == 5:
                    load_ebq(1, 1)
                    load_ebd(1, 1)
                    load_ebd(1, 0)

                attn_full, attn_diag = {}, {}
                state["attn"][(hi, c)] = (attn_full, attn_diag)

                # QK work units for this chunk, b-interleaved
                qk_units = []
                for g in range(2 * c):
                    for b in range(B):
                        qk_units.append(("full", b, g))
                for pair in range(2):
                    for b in range(B):
                        qk_units.append(("diag", b, pair))

                # interleave: spread prev chunk's 8 PV units across the
                # QK units of this chunk so PE fills ACT-drain latency
                nqk = len(qk_units)
                npv = len(prev_pv) if prev_pv else 0
                pv_i = 0
                for ui, (kind, b, idx) in enumerate(qk_units):
                    if kind == "full":
                        qk_pair(hi, b, c, idx, attn_full)
                    else:
                        qk_diag(hi, b, c, idx, attn_diag)
                    owed = (npv * (ui + 1)) // nqk
                    while pv_i < owed:
                        pv_unit(*prev_pv[pv_i], state)
                        pv_i += 1
                while prev_pv and pv_i < npv:
                    pv_unit(*prev_pv[pv_i], state)
                    pv_i += 1

                prev_pv = [
                    (hi, b, c, sub) for sub in range(JPC) for b in range(B)
                ]

            # drain: PV of the last chunk in sequence ((1,0): smallest)
            for args in prev_pv:
                pv_unit(*args, state)

    nc.finalize()
    return nc


_NC_CACHE = None


def _get_nc():
    global _NC_CACHE
    if _NC_CACHE is None:
        _NC_CACHE = build_nc()
    return _NC_CACHE


def _marshal(q, k, v, attn_bias):
    """Slice/cast/transpose the full inputs into per-core input maps."""
    qs = np.ascontiguousarray(
        np.swapaxes(q.astype(np.float32) * np.float32(SCALE), 2, 3)
    ).astype(ml_dtypes.bfloat16)
    ks = np.ascontiguousarray(np.swapaxes(k.astype(np.float32), 2, 3)).astype(
        ml_dtypes.bfloat16
    )
    # v with ones column, partition-major, halves merged:
    # [B, H, JB(p), 2(half), JPH, D+1]
    vb = v.astype(np.float32)
    vp = np.empty((B, H, N, D + 1), dtype=np.float32)
    vp[..., :D] = vb
    vp[..., D] = 1.0
    vp = vp.reshape(B, H, 2, JPH, JB, D + 1).transpose(0, 1, 4, 2, 3, 5)
    vp = np.ascontiguousarray(vp).astype(ml_dtypes.bfloat16)

    jj = np.arange(N, dtype=np.int32)[:, None]
    ii = np.arange(N, dtype=np.int32)[None, :]
    keep = jj <= ii

    in_maps = []
    for cc in range(NCORES):
        h0 = cc * HPC
        ebF = np.empty((HPC, N, N), dtype=ml_dtypes.bfloat16)
        ebD = np.empty((HPC, NCHUNK, JB, DPACK), dtype=ml_dtypes.bfloat16)
        for hh in range(HPC):
            eb = np.where(
                keep, np.exp(attn_bias[0, h0 + hh].T.astype(np.float32)), 0.0
            ).astype(ml_dtypes.bfloat16)
            ebF[hh] = eb
            for c in range(NCHUNK):
                i0 = c * CHUNK
                for kk2 in range(JPC):
                    j0 = (JPC * c + kk2) * JB
                    o = DSEG[kk2]
                    ebD[hh, c, :, o : o + DW[kk2]] = eb[
                        j0 : j0 + JB, i0 + kk2 * JB : i0 + CHUNK
                    ]
        in_maps.append(
            {
                "qT": np.ascontiguousarray(qs[:, h0 : h0 + HPC]),
                "kT": np.ascontiguousarray(ks[:, h0 : h0 + HPC]),
                "vp": vp[:, h0 : h0 + HPC].copy(),
                "ebF": ebF,
                "ebD": ebD,
            }
        )
    return in_maps


def run(q, k, v, attn_bias, trace=False):
    nc = _get_nc()
    in_maps = _marshal(q, k, v, attn_bias)
    res = run_bass_kernel_spmd(
        nc, in_maps, core_ids=list(range(NCORES)), trace=trace
    )
    out = np.empty((B, H, N, D), dtype=np.float32)
    for cc in range(NCORES):
        # [HPC, NCHUNK, JB(p), B*JPC*(D+1)] bf16
        arr = np.asarray(res.results[cc]["out"]).astype(np.float32)
        arr = arr.reshape(HPC, NCHUNK, JB, B, JPC, D + 1)
        o = arr[..., :D] / arr[..., D:]
        # [h, c, p, b, s, d] -> row i = c*512 + s*128 + p
        o = o.transpose(3, 0, 1, 4, 2, 5).reshape(B, HPC, N, D)
        out[:, cc * HPC : (cc + 1) * HPC] = o
    return out, res


def kernel(q, k, v, mask, attn_bias):
    # mask is all-ones per the input spec; the causal mask is baked into the
    # expb marshaling (zeros above the diagonal).
    out, _ = run(
        np.asarray(q), np.asarray(k), np.asarray(v), np.asarray(attn_bias)
    )
    return out


if __name__ == "__main__":
    import reference

    inputs = {kk: np.asarray(vv) for kk, vv in reference.setup_inputs().items()}
    got = kernel(**inputs)
    want = np.asarray(reference.reference(**inputs))
    denom = np.abs(want).max()
    print("abs max err:", np.abs(got - want).max())
    print("rel err:", np.abs(got - want).max() / denom)


# revision 17
# speedup vs baseline: 1.0084x; 1.0084x over previous
"""Causal attention with bias for B=2,H=16,N=2048,D=128 on 8 trn2 NeuronCores.

Sharding: core c handles heads {2c, 2c+1} for both batches (head-parallel).

Algorithm (v4, ACT-bound design):
  exp(s + bias) = exp(s) * exp(bias), with exp(bias) precomputed on the host
  (zeros above the diagonal double as the causal mask). Device per tile:
    PE:  S^T[j,i] = kT^T q  (bf16, q pre-scaled)      -> PSUM f32
    ACT: exp(S^T)                                     -> SBUF bf16
    DVE: attn = exp(S^T) * expb   (bf16, in-place)
    PE:  PV against [v | ones]  (denominator rides in column D)
    DVE: po (f32 PSUM) -> bf16 staging
  numerator/denominator division happens on the HOST (fp32), so no
  reciprocal / normalize on device.

  The scalar engine is the bottleneck (~8.9e6 exps/core at 1 elem/cycle +
  ~280ns/instr, capped at 1024-elem tiles by the 8-bank PSUM); the schedule
  keeps ACT streaming: PV of the previous chunk is interleaved between the
  QK pairs of the current chunk, head 1 runs its chunks in descending order
  so the drain tail is minimal, and DMA issues are merged into few large
  transfers so the sync sequencer never backs up.
"""

import os

import numpy as np
import ml_dtypes

import concourse.bass as bass
import concourse.bacc as bacc
import concourse.mybir as mybir
import concourse.tile as tile
from concourse.bass_utils import run_bass_kernel_spmd

B, H, N, D = 2, 16, 2048, 128
NCORES = 8
HPC = H // NCORES          # heads per core
SCALE = float(D) ** -0.5
CHUNK = 512                # i-chunk width (one psum bank of fp32)
JB = 128                   # j block (partition dim of S^T tiles)
NCHUNK = N // CHUNK        # 4
JPC = CHUNK // JB          # j blocks per chunk: 4
HALF = N // 2
JPH = HALF // JB           # j blocks per v half-tile: 8

F32 = mybir.dt.float32
BF16 = mybir.dt.bfloat16

# diag pack segment offsets for k=0..3 (widths 512,384,256,128)
DSEG = [0, 512, 896, 1152]
DW = [512, 384, 256, 128]
DPACK = 1280

PASSES_OFF = set(
    p for p in os.environ.get("ATTN_PASSES_OFF", "").split(",") if p
)


class PatchedBacc(bacc.Bacc):
    """Bacc with individually disableable scheduling passes (race bisection)."""

    def move_matmul_waits_to_ldweights(self):
        if "nomm" not in PASSES_OFF:
            super().move_matmul_waits_to_ldweights()

    def replace_nops_with_events(self):
        if "noevt" not in PASSES_OFF:
            super().replace_nops_with_events()

    def fuse_nops(self, engine):
        if "nofuse" not in PASSES_OFF:
            super().fuse_nops(engine)

    def fuse_regops(self):
        if "noregfuse" not in PASSES_OFF:
            super().fuse_regops()


def build_nc():
    nc = PatchedBacc(None, target_bir_lowering=False)

    qT_d = nc.dram_tensor("qT", [B, HPC, D, N], BF16, kind="ExternalInput").ap()
    kT_d = nc.dram_tensor("kT", [B, HPC, D, N], BF16, kind="ExternalInput").ap()
    # v with ones column, partition-major, halves merged: [b, h, p, half, jb, d+1]
    vp_d = nc.dram_tensor(
        "vp", [B, HPC, JB, 2, JPH, D + 1], BF16, kind="ExternalInput"
    ).ap()
    # exp(bias^T) full matrix (zeros above diagonal), natural [h, j, i]
    ebF_d = nc.dram_tensor("ebF", [HPC, N, N], BF16, kind="ExternalInput").ap()
    # exp(bias^T) diag blocks, packed per chunk: [h, c, p, 1280]
    ebD_d = nc.dram_tensor(
        "ebD", [HPC, NCHUNK, JB, DPACK], BF16, kind="ExternalInput"
    ).ap()
    # numerator | denominator staging: [h, c, p, b*4*(D+1)]
    out_d = nc.dram_tensor(
        "out", [HPC, NCHUNK, JB, B * JPC * (D + 1)], BF16, kind="ExternalOutput"
    ).ap()

    with tile.TileContext(nc) as tc:
        with (
            tc.tile_pool(name="singles", bufs=1) as singles,
            tc.tile_pool(name="kq", bufs=4) as kq_pool,
            tc.tile_pool(name="vp", bufs=4) as v_pool,
            tc.tile_pool(name="ebq", bufs=2) as ebq_pool,
            tc.tile_pool(name="ebd", bufs=3) as ebd_pool,
            tc.tile_pool(name="attn", bufs=28) as attn_pool,
            tc.tile_pool(name="stage", bufs=3) as stage_pool,
            tc.tile_pool(name="ps", bufs=3, space="PSUM") as ps_pool,
            tc.tile_pool(name="po", bufs=2, space="PSUM") as po_pool,
        ):
            kq_t, v_t = {}, {}

            # ---- loads ----------------------------------------------------

            def load_kq_small(hi, b, eng):
                """chunk-0 columns of qT/kT: fast-start tiles."""
                for which, src in (("q", qT_d), ("k", kT_d)):
                    t = kq_pool.tile(
                        [D, CHUNK], BF16, tag="kq0", name=f"{which}0_t"
                    )
                    eng.dma_start(out=t[:], in_=src[b, hi, :, 0:CHUNK])
                    kq_t[(which, hi, b, "c0")] = t

            def load_kq_rest(hi, b):
                """columns 512:2048 of qT/kT for head 0."""
                for which, src in (("q", qT_d), ("k", kT_d)):
                    t = kq_pool.tile(
                        [D, N - CHUNK], BF16, tag="kqr", name=f"{which}r_t"
                    )
                    nc.sync.dma_start(out=t[:], in_=src[b, hi, :, CHUNK:N])
                    kq_t[(which, hi, b, "rest")] = t

            def load_kq_full(hi, b):
                """whole rows of qT/kT for head 1."""
                for which, src in (("q", qT_d), ("k", kT_d)):
                    t = kq_pool.tile([D, N], BF16, tag="kqf", name=f"{which}f_t")
                    nc.sync.dma_start(out=t[:], in_=src[b, hi, :, :])
                    kq_t[(which, hi, b, "full")] = t

            def kq_col(which, hi, b, col0, width):
                """[D, width] slice at global column col0."""
                t = kq_t.get((which, hi, b, "full"))
                if t is not None:
                    return t[:, col0 : col0 + width]
                if col0 < CHUNK:
                    assert col0 + width <= CHUNK
                    return kq_t[(which, hi, b, "c0")][:, col0 : col0 + width]
                return kq_t[(which, hi, b, "rest")][
                    :, col0 - CHUNK : col0 - CHUNK + width
                ]

            def kT_sl(hi, b, jb):
                return kq_col("k", hi, b, jb * JB, JB)

            def qT_sl(hi, b, c, off=0):
                return kq_col("q", hi, b, c * CHUNK + off, CHUNK - off)

            def load_v(hi, b):
                t = v_pool.tile([JB, 2, JPH, D + 1], BF16, tag="v", name="v_t")
                nc.sync.dma_start(out=t[:], in_=vp_d[b, hi])
                v_t[(hi, b)] = t

            def v_sl(hi, b, jb):
                return v_t[(hi, b)][:, jb // JPH, jb % JPH, :]

            ebq_tiles, ebd_tiles = {}, {}

            def load_ebq(hi, c, eng=None):
                """full-region expb for chunk (hi, c): one DMA, 4c j-blocks."""
                if c == 0:
                    return
                i0 = c * CHUNK
                t = ebq_pool.tile(
                    [JB, 4 * NCHUNK - 4, CHUNK], BF16, tag="ebq", name="ebq_t"
                )
                (eng or nc.sync).dma_start(
                    out=t[:, 0 : 4 * c, :],
                    in_=ebF_d[hi, 0 : c * CHUNK, i0 : i0 + CHUNK].rearrange(
                        "(t p) i -> p t i", p=JB
                    ),
                )
                ebq_tiles[(hi, c)] = t

            def load_ebd(hi, c, eng=None):
                t = ebd_pool.tile([JB, DPACK], BF16, tag="ebd", name="ebd_t")
                (eng or nc.sync).dma_start(out=t[:], in_=ebD_d[hi, c])
                ebd_tiles[(hi, c)] = t

            # ---- per-(hi, chunk) work units -------------------------------

            def qk_pair(hi, b, c, g, attn_full):
                """Full pair g (jb = 2g, 2g+1): QK -> exp -> *expb."""
                ps = ps_pool.tile([JB, 2 * CHUNK], F32, tag="ps", name="ps_t")
                for t in range(2):
                    sl = slice(t * CHUNK, (t + 1) * CHUNK)
                    nc.tensor.matmul(
                        ps[:, sl],
                        lhsT=kT_sl(hi, b, 2 * g + t),
                        rhs=qT_sl(hi, b, c),
                        start=True,
                        stop=True,
                    )
                at = attn_pool.tile([JB, 2 * CHUNK], BF16, tag="attn", name="at_t")
                nc.scalar.activation(
                    at[:], ps[:], mybir.ActivationFunctionType.Exp
                )
                eb = ebq_tiles[(hi, c)][:, 2 * g : 2 * g + 2, :]
                nc.vector.tensor_mul(
                    at[:], at[:], eb.rearrange("p t i -> p (t i)")
                )
                attn_full[(b, g)] = at

            def qk_diag(hi, b, c, pair, attn_diag):
                """Diag pair (k = 2*pair, 2*pair+1): narrowed QK, exp, *expb."""
                ebd = ebd_tiles[(hi, c)]
                ps = ps_pool.tile([JB, 2 * CHUNK], F32, tag="ps", name="ps_t")
                for t in range(2):
                    k = 2 * pair + t
                    off = k * JB
                    nc.tensor.matmul(
                        ps[:, t * CHUNK + off : (t + 1) * CHUNK],
                        lhsT=kT_sl(hi, b, JPC * c + k),
                        rhs=qT_sl(hi, b, c, off),
                        start=True,
                        stop=True,
                    )
                at = attn_pool.tile([JB, 2 * CHUNK], BF16, tag="attn", name="at_t")
                if pair == 0:
                    # k=0 is full width; k=1 wastes only 128 cols: one big
                    # activation beats two narrowed ones (fixed ~280ns/instr)
                    nc.scalar.activation(
                        at[:], ps[:], mybir.ActivationFunctionType.Exp
                    )
                else:
                    for t in range(2):
                        k = 2 * pair + t
                        off = k * JB
                        sl = slice(t * CHUNK + off, (t + 1) * CHUNK)
                        nc.scalar.activation(
                            at[:, sl], ps[:, sl],
                            mybir.ActivationFunctionType.Exp,
                        )
                for t in range(2):
                    k = 2 * pair + t
                    off = k * JB
                    sl = slice(t * CHUNK + off, (t + 1) * CHUNK)
                    nc.vector.tensor_mul(
                        at[:, sl], at[:, sl],
                        ebd[:, DSEG[k] : DSEG[k] + DW[k]],
                    )
                attn_diag[(b, pair)] = at

            def attn_slice(b, c, jb, sub, attn_full, attn_diag):
                if jb < JPC * c:
                    t = attn_full[(b, jb // 2)]
                    o = (jb % 2) * CHUNK
                else:
                    k = jb - JPC * c
                    t = attn_diag[(b, k // 2)]
                    o = (k % 2) * CHUNK
                return t[:, o + sub * JB : o + (sub + 1) * JB]

            def pv_unit(hi, b, c, sub, state):
                """PV accumulation for output block ib = 4c+sub of (b, c)."""
                ib = JPC * c + sub
                po = po_pool.tile([JB, D + 1], F32, tag="po", name="po_t")
                af, ad = state["attn"][(hi, c)]
                for jb in range(ib + 1):
                    nc.tensor.matmul(
                        po[:],
                        lhsT=attn_slice(b, c, jb, sub, af, ad),
                        rhs=v_sl(hi, b, jb),
                        start=(jb == 0),
                        stop=(jb == ib),
                    )
                stg = state["stg"].get((hi, c))
                if stg is None:
                    stg = stage_pool.tile(
                        [JB, B * JPC * (D + 1)], BF16, tag="stg", name="stg_t"
                    )
                    state["stg"][(hi, c)] = stg
                o = (b * JPC + sub) * (D + 1)
                nc.vector.tensor_copy(out=stg[:, o : o + (D + 1)], in_=po[:])
                if sub == JPC - 1 and b == B - 1:
                    nc.sync.dma_start(out=out_d[hi, c], in_=stg[:])

            # ---- main schedule -------------------------------------------
            # head 0 ascending, head 1 descending: the final chunk is then
            # (1, 0), whose PV drain is the cheapest possible tail
            seq = [(0, 0), (0, 1), (0, 2), (0, 3), (1, 3), (1, 2), (1, 1), (1, 0)]

            state = {"attn": {}, "stg": {}}
            prev_pv = None

            for ti, (hi, c) in enumerate(seq):
                if ti == 0:
                    # DMA transfers effectively serialize in issue order, so
                    # startup criticals go first, all on the scalar engine's
                    # queue (its preamble finishes ~3us before sync's):
                    # chunk-0 q/k tiles, then the first chunk's expb.
                    for b in range(B):
                        load_kq_small(0, b, nc.scalar)
                    # warm-up exp AFTER the DMA issues so it doesn't block
                    warm = singles.tile([JB, 1], F32, tag="warm", name="warm")
                    nc.vector.memset(warm[:], 0.0)
                    nc.scalar.activation(
                        warm[:], warm[:], mybir.ActivationFunctionType.Exp
                    )
                    load_ebd(0, 0, eng=nc.scalar)
                    load_ebq(0, 1, eng=nc.scalar)
                    # gate the sync queue behind the scalar-issued criticals:
                    # tiny SBUF->SBUF copies depending on the last kq tile.
                    # The sync sequencer's wait-queue depth is 4, so FIVE
                    # dependent gates fully serialize everything behind them —
                    # sync's big, less urgent transfers then can't steal DMA
                    # bandwidth from the startup criticals.
                    gate = singles.tile([1, 10], BF16, tag="gate", name="gate")
                    for gi in range(5):
                        nc.sync.dma_start(
                            out=gate[0:1, 2 * gi : 2 * gi + 2],
                            in_=kq_t[("k", 0, 1, "c0")][0:1, 0:2],
                        )
                    # remaining loads in need-order on sync
                    for b in range(B):
                        load_kq_rest(0, b)
                    for b in range(B):
                        load_v(0, b)
                    load_ebd(0, 1)
                elif ti == 1:
                    load_ebq(0, 2)
                    load_ebd(0, 2)
                    for b in range(B):
                        load_kq_full(1, b)
                elif ti == 2:
                    load_ebq(0, 3)
                    load_ebd(0, 3)
                    for b in range(B):
                        load_v(1, b)
                elif ti == 3:
                    load_ebq(1, 3)
                    load_ebd(1, 3)
                elif ti == 4:
                    load_ebq(1, 2)
                    load_ebd(1, 2)
                elif ti == 5:
                    load_ebq(1, 1)
                    load_ebd(1, 1)
                    load_ebd(1, 0)

                attn_full, attn_diag = {}, {}
                state["attn"][(hi, c)] = (attn_full, attn_diag)

                # QK work units for this chunk, b-interleaved
                qk_units = []
                for g in range(2 * c):
                    for b in range(B):
                        qk_units.append(("full", b, g))
                for pair in range(2):
                    for b in range(B):
                        qk_units.append(("diag", b, pair))

                # interleave: spread prev chunk's 8 PV units across the
                # QK units of this chunk so PE fills ACT-drain latency
                nqk = len(qk_units)
                npv = len(prev_pv) if prev_pv else 0
                pv_i = 0
                for ui, (kind, b, idx) in enumerate(qk_units):
                    if kind == "full":
                        qk_pair(hi, b, c, idx, attn_full)
                    else:
                        qk_diag(hi, b, c, idx, attn_diag)
                    owed = (npv * (ui + 1)) // nqk
                    while pv_i < owed:
                        pv_unit(*prev_pv[pv_i], state)
                        pv_i += 1
                while prev_pv and pv_i < npv:
                    pv_unit(*prev_pv[pv_i], state)
                    pv_i += 1

                prev_pv = [
                    (hi, b, c, sub) for sub in range(JPC) for b in range(B)
                ]

            # drain: PV of the last chunk in sequence ((1,0): smallest)
            for args in prev_pv:
                pv_unit(*args, state)

    nc.finalize()
    return nc


_NC_CACHE = None


def _get_nc():
    global _NC_CACHE
    if _NC_CACHE is None:
        _NC_CACHE = build_nc()
    return _NC_CACHE


def _marshal(q, k, v, attn_bias):
    """Slice/cast/transpose the full inputs into per-core input maps."""
    qs = np.ascontiguousarray(
        np.swapaxes(q.astype(np.float32) * np.float32(SCALE), 2, 3)
    ).astype(ml_dtypes.bfloat16)
    ks = np.ascontiguousarray(np.swapaxes(k.astype(np.float32), 2, 3)).astype(
        ml_dtypes.bfloat16
    )
    # v with ones column, partition-major, halves merged:
    # [B, H, JB(p), 2(half), JPH, D+1]
    vb = v.astype(np.float32)
    vp = np.empty((B, H, N, D + 1), dtype=np.float32)
    vp[..., :D] = vb
    vp[..., D] = 1.0
    vp = vp.reshape(B, H, 2, JPH, JB, D + 1).transpose(0, 1, 4, 2, 3, 5)
    vp = np.ascontiguousarray(vp).astype(ml_dtypes.bfloat16)

    jj = np.arange(N, dtype=np.int32)[:, None]
    ii = np.arange(N, dtype=np.int32)[None, :]
    keep = jj <= ii

    in_maps = []
    for cc in range(NCORES):
        h0 = cc * HPC
        ebF = np.empty((HPC, N, N), dtype=ml_dtypes.bfloat16)
        ebD = np.empty((HPC, NCHUNK, JB, DPACK), dtype=ml_dtypes.bfloat16)
        for hh in range(HPC):
            eb = np.where(
                keep, np.exp(attn_bias[0, h0 + hh].T.astype(np.float32)), 0.0
            ).astype(ml_dtypes.bfloat16)
            ebF[hh] = eb
            for c in range(NCHUNK):
                i0 = c * CHUNK
                for kk2 in range(JPC):
                    j0 = (JPC * c + kk2) * JB
                    o = DSEG[kk2]
                    ebD[hh, c, :, o : o + DW[kk2]] = eb[
                        j0 : j0 + JB, i0 + kk2 * JB : i0 + CHUNK
                    ]
        in_maps.append(
            {
                "qT": np.ascontiguousarray(qs[:, h0 : h0 + HPC]),
                "kT": np.ascontiguousarray(ks[:, h0 : h0 + HPC]),
                "vp": vp[:, h0 : h0 + HPC].copy(),
                "ebF": ebF,
                "ebD": ebD,
            }
        )
    return in_maps


def run(q, k, v, attn_bias, trace=False):
    nc = _get_nc()
    in_maps = _marshal(q, k, v, attn_bias)
    res = run_bass_kernel_spmd(
        nc, in_maps, core_ids=list(range(NCORES)), trace=trace
    )
    out = np.empty((B, H, N, D), dtype=np.float32)
    for cc in range(NCORES):
        # [HPC, NCHUNK, JB(p), B*JPC*(D+1)] bf16
        arr = np.asarray(res.results[cc]["out"]).astype(np.float32)
        arr = arr.reshape(HPC, NCHUNK, JB, B, JPC, D + 1)
        o = arr[..., :D] / arr[..., D:]
        # [h, c, p, b, s, d] -> row i = c*512 + s*128 + p
        o = o.transpose(3, 0, 1, 4, 2, 5).reshape(B, HPC, N, D)
        out[:, cc * HPC : (cc + 1) * HPC] = o
    return out, res


def kernel(q, k, v, mask, attn_bias):
    # mask is all-ones per the input spec; the causal mask is baked into the
    # expb marshaling (zeros above the diagonal).
    out, _ = run(
        np.asarray(q), np.asarray(k), np.asarray(v), np.asarray(attn_bias)
    )
    return out


if __name__ == "__main__":
    import reference

    inputs = {kk: np.asarray(vv) for kk, vv in reference.setup_inputs().items()}
    got = kernel(**inputs)
    want = np.asarray(reference.reference(**inputs))
    denom = np.abs(want).max()
    print("abs max err:", np.abs(got - want).max())
    print("rel err:", np.abs(got - want).max() / denom)


# revision 20
# speedup vs baseline: 1.0135x; 1.0051x over previous
"""Causal attention with bias for B=2,H=16,N=2048,D=128 on 8 trn2 NeuronCores.

Sharding: core c handles heads {2c, 2c+1} for both batches (head-parallel).

Algorithm (v4, ACT-bound design):
  exp(s + bias) = exp(s) * exp(bias), with exp(bias) precomputed on the host
  (zeros above the diagonal double as the causal mask). Device per tile:
    PE:  S^T[j,i] = kT^T q  (bf16, q pre-scaled)      -> PSUM f32
    ACT: exp(S^T)                                     -> SBUF bf16
    DVE: attn = exp(S^T) * expb   (bf16, in-place)
    PE:  PV against [v | ones]  (denominator rides in column D)
    DVE: po (f32 PSUM) -> bf16 staging
  numerator/denominator division happens on the HOST (fp32), so no
  reciprocal / normalize on device.

  The scalar engine is the bottleneck (~8.9e6 exps/core at 1 elem/cycle +
  ~280ns/instr, capped at 1024-elem tiles by the 8-bank PSUM); the schedule
  keeps ACT streaming: PV of the previous chunk is interleaved between the
  QK pairs of the current chunk, head 1 runs its chunks in descending order
  so the drain tail is minimal, and DMA issues are merged into few large
  transfers so the sync sequencer never backs up.
"""

import os

import numpy as np
import ml_dtypes

import concourse.bass as bass
import concourse.bacc as bacc
import concourse.mybir as mybir
import concourse.tile as tile
from concourse.bass_utils import run_bass_kernel_spmd

B, H, N, D = 2, 16, 2048, 128
NCORES = 8
HPC = H // NCORES          # heads per core
SCALE = float(D) ** -0.5
CHUNK = 512                # i-chunk width (one psum bank of fp32)
JB = 128                   # j block (partition dim of S^T tiles)
NCHUNK = N // CHUNK        # 4
JPC = CHUNK // JB          # j blocks per chunk: 4
HALF = N // 2
JPH = HALF // JB           # j blocks per v half-tile: 8

F32 = mybir.dt.float32
BF16 = mybir.dt.bfloat16

# diag pack segment offsets for k=0..3 (widths 512,384,256,128)
DSEG = [0, 512, 896, 1152]
DW = [512, 384, 256, 128]
DPACK = 1280

PASSES_OFF = set(
    p for p in os.environ.get("ATTN_PASSES_OFF", "").split(",") if p
)


class PatchedBacc(bacc.Bacc):
    """Bacc with individually disableable scheduling passes (race bisection)."""

    def move_matmul_waits_to_ldweights(self):
        if "nomm" not in PASSES_OFF:
            super().move_matmul_waits_to_ldweights()

    def replace_nops_with_events(self):
        if "noevt" not in PASSES_OFF:
            super().replace_nops_with_events()

    def fuse_nops(self, engine):
        if "nofuse" not in PASSES_OFF:
            super().fuse_nops(engine)

    def fuse_regops(self):
        if "noregfuse" not in PASSES_OFF:
            super().fuse_regops()


def build_nc():
    nc = PatchedBacc(None, target_bir_lowering=False)

    qT_d = nc.dram_tensor("qT", [B, HPC, D, N], BF16, kind="ExternalInput").ap()
    kT_d = nc.dram_tensor("kT", [B, HPC, D, N], BF16, kind="ExternalInput").ap()
    # v with ones column, partition-major, halves merged: [b, h, p, half, jb, d+1]
    vp_d = nc.dram_tensor(
        "vp", [B, HPC, JB, 2, JPH, D + 1], BF16, kind="ExternalInput"
    ).ap()
    # exp(bias^T) full matrix (zeros above diagonal), natural [h, j, i]
    ebF_d = nc.dram_tensor("ebF", [HPC, N, N], BF16, kind="ExternalInput").ap()
    # exp(bias^T) diag blocks, packed per chunk: [h, c, p, 1280]
    ebD_d = nc.dram_tensor(
        "ebD", [HPC, NCHUNK, JB, DPACK], BF16, kind="ExternalInput"
    ).ap()
    # numerator | denominator staging: [h, c, p, b*4*(D+1)]
    out_d = nc.dram_tensor(
        "out", [HPC, NCHUNK, JB, B * JPC * (D + 1)], BF16, kind="ExternalOutput"
    ).ap()

    with tile.TileContext(nc) as tc:
        with (
            tc.tile_pool(name="singles", bufs=1) as singles,
            tc.tile_pool(name="kq", bufs=4) as kq_pool,
            tc.tile_pool(name="vp", bufs=4) as v_pool,
            tc.tile_pool(name="ebq", bufs=2) as ebq_pool,
            tc.tile_pool(name="ebd", bufs=3) as ebd_pool,
            tc.tile_pool(name="attn", bufs=28) as attn_pool,
            tc.tile_pool(name="stage", bufs=3) as stage_pool,
            tc.tile_pool(name="ps", bufs=3, space="PSUM") as ps_pool,
            tc.tile_pool(name="po", bufs=2, space="PSUM") as po_pool,
        ):
            kq_t, v_t = {}, {}

            # ---- loads ----------------------------------------------------

            def load_kq_small(hi, b, eng):
                """chunk-0 columns of qT/kT: fast-start tiles."""
                for which, src in (("q", qT_d), ("k", kT_d)):
                    t = kq_pool.tile(
                        [D, CHUNK], BF16, tag="kq0", name=f"{which}0_t"
                    )
                    eng.dma_start(out=t[:], in_=src[b, hi, :, 0:CHUNK])
                    kq_t[(which, hi, b, "c0")] = t

            def load_kq_rest(hi, b):
                """columns 512:2048 of qT/kT for head 0."""
                for which, src in (("q", qT_d), ("k", kT_d)):
                    t = kq_pool.tile(
                        [D, N - CHUNK], BF16, tag="kqr", name=f"{which}r_t"
                    )
                    nc.sync.dma_start(out=t[:], in_=src[b, hi, :, CHUNK:N])
                    kq_t[(which, hi, b, "rest")] = t

            def load_kq_full(hi, b):
                """whole rows of qT/kT for head 1."""
                for which, src in (("q", qT_d), ("k", kT_d)):
                    t = kq_pool.tile([D, N], BF16, tag="kqf", name=f"{which}f_t")
                    nc.sync.dma_start(out=t[:], in_=src[b, hi, :, :])
                    kq_t[(which, hi, b, "full")] = t

            def kq_col(which, hi, b, col0, width):
                """[D, width] slice at global column col0."""
                t = kq_t.get((which, hi, b, "full"))
                if t is not None:
                    return t[:, col0 : col0 + width]
                if col0 < CHUNK:
                    assert col0 + width <= CHUNK
                    return kq_t[(which, hi, b, "c0")][:, col0 : col0 + width]
                return kq_t[(which, hi, b, "rest")][
                    :, col0 - CHUNK : col0 - CHUNK + width
                ]

            def kT_sl(hi, b, jb):
                return kq_col("k", hi, b, jb * JB, JB)

            def qT_sl(hi, b, c, off=0):
                return kq_col("q", hi, b, c * CHUNK + off, CHUNK - off)

            def load_v(hi, b):
                t = v_pool.tile([JB, 2, JPH, D + 1], BF16, tag="v", name="v_t")
                nc.sync.dma_start(out=t[:], in_=vp_d[b, hi])
                v_t[(hi, b)] = t

            def v_sl(hi, b, jb):
                return v_t[(hi, b)][:, jb // JPH, jb % JPH, :]

            ebq_tiles, ebd_tiles = {}, {}

            def load_ebq(hi, c, eng=None):
                """full-region expb for chunk (hi, c): one DMA, 4c j-blocks."""
                if c == 0:
                    return
                i0 = c * CHUNK
                t = ebq_pool.tile(
                    [JB, 4 * NCHUNK - 4, CHUNK], BF16, tag="ebq", name="ebq_t"
                )
                (eng or nc.sync).dma_start(
                    out=t[:, 0 : 4 * c, :],
                    in_=ebF_d[hi, 0 : c * CHUNK, i0 : i0 + CHUNK].rearrange(
                        "(t p) i -> p t i", p=JB
                    ),
                )
                ebq_tiles[(hi, c)] = t

            def load_ebd(hi, c, eng=None):
                t = ebd_pool.tile([JB, DPACK], BF16, tag="ebd", name="ebd_t")
                (eng or nc.sync).dma_start(out=t[:], in_=ebD_d[hi, c])
                ebd_tiles[(hi, c)] = t

            # ---- per-(hi, chunk) work units -------------------------------

            def qk_pair(hi, b, c, g, attn_full):
                """Full pair g (jb = 2g, 2g+1): QK -> exp -> *expb."""
                ps = ps_pool.tile([JB, 2 * CHUNK], F32, tag="ps", name="ps_t")
                for t in range(2):
                    sl = slice(t * CHUNK, (t + 1) * CHUNK)
                    nc.tensor.matmul(
                        ps[:, sl],
                        lhsT=kT_sl(hi, b, 2 * g + t),
                        rhs=qT_sl(hi, b, c),
                        start=True,
                        stop=True,
                    )
                at = attn_pool.tile([JB, 2 * CHUNK], BF16, tag="attn", name="at_t")
                nc.scalar.activation(
                    at[:], ps[:], mybir.ActivationFunctionType.Exp
                )
                eb = ebq_tiles[(hi, c)][:, 2 * g : 2 * g + 2, :]
                nc.vector.tensor_mul(
                    at[:], at[:], eb.rearrange("p t i -> p (t i)")
                )
                attn_full[(b, g)] = at

            def qk_diag(hi, b, c, pair, attn_diag):
                """Diag pair (k = 2*pair, 2*pair+1): narrowed QK, exp, *expb."""
                ebd = ebd_tiles[(hi, c)]
                ps = ps_pool.tile([JB, 2 * CHUNK], F32, tag="ps", name="ps_t")
                for t in range(2):
                    k = 2 * pair + t
                    off = k * JB
                    nc.tensor.matmul(
                        ps[:, t * CHUNK + off : (t + 1) * CHUNK],
                        lhsT=kT_sl(hi, b, JPC * c + k),
                        rhs=qT_sl(hi, b, c, off),
                        start=True,
                        stop=True,
                    )
                at = attn_pool.tile([JB, 2 * CHUNK], BF16, tag="attn", name="at_t")
                if pair == 0:
                    # k=0 is full width; k=1 wastes only 128 cols: one big
                    # activation beats two narrowed ones (fixed ~280ns/instr)
                    nc.scalar.activation(
                        at[:], ps[:], mybir.ActivationFunctionType.Exp
                    )
                else:
                    for t in range(2):
                        k = 2 * pair + t
                        off = k * JB
                        sl = slice(t * CHUNK + off, (t + 1) * CHUNK)
                        nc.scalar.activation(
                            at[:, sl], ps[:, sl],
                            mybir.ActivationFunctionType.Exp,
                        )
                for t in range(2):
                    k = 2 * pair + t
                    off = k * JB
                    sl = slice(t * CHUNK + off, (t + 1) * CHUNK)
                    nc.vector.tensor_mul(
                        at[:, sl], at[:, sl],
                        ebd[:, DSEG[k] : DSEG[k] + DW[k]],
                    )
                attn_diag[(b, pair)] = at

            def attn_slice(b, c, jb, sub, attn_full, attn_diag):
                if jb < JPC * c:
                    t = attn_full[(b, jb // 2)]
                    o = (jb % 2) * CHUNK
                else:
                    k = jb - JPC * c
                    t = attn_diag[(b, k // 2)]
                    o = (k % 2) * CHUNK
                return t[:, o + sub * JB : o + (sub + 1) * JB]

            def make_pv_units(hi, c):
                """PV work for chunk (hi, c), split into resumable units."""
                return [
                    {"hi": hi, "b": b, "c": c, "sub": sub, "po": None, "jb": 0}
                    for sub in range(JPC)
                    for b in range(B)
                ]

            def pv_advance(u, budget, state):
                """Issue up to `budget` PV matmuls of unit u (resumable: the
                PSUM accumulation survives interleaved QK matmuls).  Fires
                the cast/output DMA when the unit completes.  Returns the
                number of matmuls issued."""
                hi, b, c, sub = u["hi"], u["b"], u["c"], u["sub"]
                ib = JPC * c + sub
                if u["po"] is None:
                    u["po"] = po_pool.tile([JB, D + 1], F32, tag="po", name="po_t")
                af, ad = state["attn"][(hi, c)]
                issued = 0
                while u["jb"] <= ib and issued < budget:
                    jb = u["jb"]
                    nc.tensor.matmul(
                        u["po"][:],
                        lhsT=attn_slice(b, c, jb, sub, af, ad),
                        rhs=v_sl(hi, b, jb),
                        start=(jb == 0),
                        stop=(jb == ib),
                        skip_group_check=True,
                    )
                    u["jb"] += 1
                    issued += 1
                if u["jb"] > ib:
                    stg = state["stg"].get((hi, c))
                    if stg is None:
                        stg = stage_pool.tile(
                            [JB, B * JPC * (D + 1)], BF16, tag="stg",
                            name="stg_t",
                        )
                        state["stg"][(hi, c)] = stg
                    o = (b * JPC + sub) * (D + 1)
                    nc.vector.tensor_copy(
                        out=stg[:, o : o + (D + 1)], in_=u["po"][:]
                    )
                    if sub == JPC - 1 and b == B - 1:
                        nc.sync.dma_start(out=out_d[hi, c], in_=stg[:])
                return issued

            # ---- main schedule -------------------------------------------
            # head 0 ascending, head 1 descending: the final chunk is then
            # (1, 0), whose PV drain is the cheapest possible tail
            seq = [(0, 0), (0, 1), (0, 2), (0, 3), (1, 3), (1, 2), (1, 1), (1, 0)]

            state = {"attn": {}, "stg": {}}
            prev_pv = None

            for ti, (hi, c) in enumerate(seq):
                if ti == 0:
                    # DMA transfers effectively serialize in issue order, so
                    # startup criticals go first, all on the scalar engine's
                    # queue (its preamble finishes ~3us before sync's):
                    # chunk-0 q/k tiles, then the first chunk's expb.
                    for b in range(B):
                        load_kq_small(0, b, nc.scalar)
                    # warm-up exp AFTER the DMA issues so it doesn't block
                    warm = singles.tile([JB, 1], F32, tag="warm", name="warm")
                    nc.vector.memset(warm[:], 0.0)
                    nc.scalar.activation(
                        warm[:], warm[:], mybir.ActivationFunctionType.Exp
                    )
                    load_ebd(0, 0, eng=nc.scalar)
                    load_ebq(0, 1, eng=nc.scalar)
                    # remaining loads in need-order on sync
                    for b in range(B):
                        load_kq_rest(0, b)
                    for b in range(B):
                        load_v(0, b)
                    load_ebd(0, 1)
                elif ti == 1:
                    load_ebq(0, 2)
                    load_ebd(0, 2)
                    for b in range(B):
                        load_kq_full(1, b)
                elif ti == 2:
                    load_ebq(0, 3)
                    load_ebd(0, 3)
                    for b in range(B):
                        load_v(1, b)
                elif ti == 3:
                    load_ebq(1, 3)
                    load_ebd(1, 3)
                elif ti == 4:
                    load_ebq(1, 2)
                    load_ebd(1, 2)
                elif ti == 5:
                    load_ebq(1, 1)
                    load_ebd(1, 1)
                    load_ebd(1, 0)

                attn_full, attn_diag = {}, {}
                state["attn"][(hi, c)] = (attn_full, attn_diag)

                # QK work units for this chunk, b-interleaved
                qk_units = []
                for g in range(2 * c):
                    for b in range(B):
                        qk_units.append(("full", b, g))
                for pair in range(2):
                    for b in range(B):
                        qk_units.append(("diag", b, pair))

                # interleave: spread the prev chunk's PV matmuls evenly
                # across this chunk's QK units (at individual-matmul
                # granularity) so PE never lumps multi-us PV batches that
                # would starve ACT of fresh QK results
                nqk = len(qk_units)
                total_mm = (
                    sum(JPC * u["c"] + u["sub"] + 1 for u in prev_pv)
                    if prev_pv
                    else 0
                )
                issued = 0
                uidx = 0
                for ui, (kind, b, idx) in enumerate(qk_units):
                    if kind == "full":
                        qk_pair(hi, b, c, idx, attn_full)
                    else:
                        qk_diag(hi, b, c, idx, attn_diag)
                    target = (total_mm * (ui + 1)) // nqk
                    while issued < target and uidx < len(prev_pv):
                        n = pv_advance(prev_pv[uidx], target - issued, state)
                        issued += n
                        if prev_pv[uidx]["jb"] > JPC * prev_pv[uidx]["c"] + prev_pv[uidx]["sub"]:
                            uidx += 1
                while prev_pv and uidx < len(prev_pv):
                    n = pv_advance(prev_pv[uidx], 1 << 30, state)
                    issued += n
                    uidx += 1

                prev_pv = make_pv_units(hi, c)

            # drain: PV of the last chunk in sequence ((1,0): smallest)
            for u in prev_pv:
                pv_advance(u, 1 << 30, state)

    nc.finalize()
    return nc


_NC_CACHE = None


def _get_nc():
    global _NC_CACHE
    if _NC_CACHE is None:
        _NC_CACHE = build_nc()
    return _NC_CACHE


def _marshal(q, k, v, attn_bias):
    """Slice/cast/transpose the full inputs into per-core input maps."""
    qs = np.ascontiguousarray(
        np.swapaxes(q.astype(np.float32) * np.float32(SCALE), 2, 3)
    ).astype(ml_dtypes.bfloat16)
    ks = np.ascontiguousarray(np.swapaxes(k.astype(np.float32), 2, 3)).astype(
        ml_dtypes.bfloat16
    )
    # v with ones column, partition-major, halves merged:
    # [B, H, JB(p), 2(half), JPH, D+1]
    vb = v.astype(np.float32)
    vp = np.empty((B, H, N, D + 1), dtype=np.float32)
    vp[..., :D] = vb
    vp[..., D] = 1.0
    vp = vp.reshape(B, H, 2, JPH, JB, D + 1).transpose(0, 1, 4, 2, 3, 5)
    vp = np.ascontiguousarray(vp).astype(ml_dtypes.bfloat16)

    jj = np.arange(N, dtype=np.int32)[:, None]
    ii = np.arange(N, dtype=np.int32)[None, :]
    keep = jj <= ii

    in_maps = []
    for cc in range(NCORES):
        h0 = cc * HPC
        ebF = np.empty((HPC, N, N), dtype=ml_dtypes.bfloat16)
        ebD = np.empty((HPC, NCHUNK, JB, DPACK), dtype=ml_dtypes.bfloat16)
        for hh in range(HPC):
            eb = np.where(
                keep, np.exp(attn_bias[0, h0 + hh].T.astype(np.float32)), 0.0
            ).astype(ml_dtypes.bfloat16)
            ebF[hh] = eb
            for c in range(NCHUNK):
                i0 = c * CHUNK
                for kk2 in range(JPC):
                    j0 = (JPC * c + kk2) * JB
                    o = DSEG[kk2]
                    ebD[hh, c, :, o : o + DW[kk2]] = eb[
                        j0 : j0 + JB, i0 + kk2 * JB : i0 + CHUNK
                    ]
        in_maps.append(
            {
                "qT": np.ascontiguousarray(qs[:, h0 : h0 + HPC]),
                "kT": np.ascontiguousarray(ks[:, h0 : h0 + HPC]),
                "vp": vp[:, h0 : h0 + HPC].copy(),
                "ebF": ebF,
                "ebD": ebD,
            }
        )
    return in_maps


def run(q, k, v, attn_bias, trace=False):
    nc = _get_nc()
    in_maps = _marshal(q, k, v, attn_bias)
    res = run_bass_kernel_spmd(
        nc, in_maps, core_ids=list(range(NCORES)), trace=trace
    )
    out = np.empty((B, H, N, D), dtype=np.float32)
    for cc in range(NCORES):
        # [HPC, NCHUNK, JB(p), B*JPC*(D+1)] bf16
        arr = np.asarray(res.results[cc]["out"]).astype(np.float32)
        arr = arr.reshape(HPC, NCHUNK, JB, B, JPC, D + 1)
        o = arr[..., :D] / arr[..., D:]
        # [h, c, p, b, s, d] -> row i = c*512 + s*128 + p
        o = o.transpose(3, 0, 1, 4, 2, 5).reshape(B, HPC, N, D)
        out[:, cc * HPC : (cc + 1) * HPC] = o
    return out, res


def kernel(q, k, v, mask, attn_bias):
    # mask is all-ones per the input spec; the causal mask is baked into the
    # expb marshaling (zeros above the diagonal).
    out, _ = run(
        np.asarray(q), np.asarray(k), np.asarray(v), np.asarray(attn_bias)
    )
    return out


if __name__ == "__main__":
    import reference

    inputs = {kk: np.asarray(vv) for kk, vv in reference.setup_inputs().items()}
    got = kernel(**inputs)
    want = np.asarray(reference.reference(**inputs))
    denom = np.abs(want).max()
    print("abs max err:", np.abs(got - want).max())
    print("rel err:", np.abs(got - want).max() / denom)
